# revision 9
# baseline (speedup 1.0000x reference)
"""BinLinear (BatchNorm -> sign-binarize -> scaled binary linear) on 8 TRN2
NeuronCores.

Reference computation (fp32, full batch):
    mean/var over batch axis of x [8192, 4096]
    h  = (x - mean) * rsqrt(var + eps) * gamma + beta          [8192, 4096]
    hb = sign(h)
    out = (hb @ W.T) * alpha[None, :] * mean_abs(h, axis=1)    [8192, 4096]

Distribution: data-parallel over the batch (1024 rows per core); a ~64 KB
AllReduce (two pipelined 32 KB chunks) produces full-batch BN statistics.

Device-side structure (per core, xT layout [feat, batch] so features sit on
SBUF partitions).  The critical path is AR-A -> signs -> matmul; everything
else (|h|, beta, BN-B coefficients) hides under it:
  phase 1  x streams in on TWO DMA queues (sync: even k-tiles, gpsimd: odd);
           per-feature sum on DVE, sum-of-squares on ACT.  Emitted in two
           halves so the chunk-A AllReduce staging + trigger land on their
           queues ahead of the second half's loads.  Tiles 0..11 stay
           resident; the rest are re-fetched for phase 2.
  AllReduce chunk A = k-tiles 0..15 (all the fp8 pair tiles) fires as soon
           as their stats land; chunk B = 16..31 follows on the CC stream.
           The BN coefficient math is emitted separately from the staging so
           the ACT Sqrt never blocks phase-1 stats or the sign chains.
  phase 2  hb = Sign(a*x + b) on ACT (fp8, exact +-1).  |h| = Abs(a*x+b) on
           ACT strictly after the signs it would delay; per-feature |h|
           accumulates across k-tiles on DVE (bf16) and one ones-matmul pair
           at the end gives the per-row beta -- no per-tile beta matmuls.
  phase 3  out = hb.T @ W, k-tiles 0..15 as fp8 DoubleRow pairs (K=256 per
           instruction), 16..31 in bf16.  While AR-B / chunk-B signs are in
           flight, a pair-only pre-stage runs the 8 DoubleRow matmuls for
           oq0..3 x all bt and spills the partials to SBUF in bf16; the main
           stream re-adds them and runs only the bf16 k-tiles for those oq.
           W is pre-scaled by alpha and 2^12 on the host (fp8 subnormal
           floor); the 2^-12 is folded into the beta scale.  Output tiles
           that complete before beta does are drained unscaled and scaled
           once beta lands.
"""

import numpy as np
import ml_dtypes

import concourse.bacc as bacc
import concourse.mybir as mybir
import concourse.tile as tile
from concourse.bass_utils import run_bass_kernel_spmd

dt = mybir.dt
AF = mybir.ActivationFunctionType
ALU = mybir.AluOpType
PM = mybir.MatmulPerfMode

N_CORES = 8
B, D = 8192, 4096          # batch, features (D_in == D_out == 4096)
BS = B // N_CORES          # 1024 batch rows per core
KT = D // 128              # 32 k-tiles (features / partitions)
EPS = 1e-5

FP8_KT = 16                # k-tiles 0..15 go through fp8 DoubleRow pairs
NPAIR = FP8_KT // 2        # 8 pair tiles
NB16 = KT - FP8_KT         # 16 bf16 k-tiles
W_SCALE = 4096.0           # 2^12 upscale for fp8 W (folded out via beta)

NRES = 12                  # k-tiles 0..11 stay resident in SBUF
PRE_OQ = 3                 # oq 0..2 get the pair-only pre-stage
NBT = BS // 128            # 8 batch tiles
NOQ = D // 512             # 8 output quarters

TRACE = False              # set by test.py for profiling runs
LAST_RESULT = None

_nc_cache = None


def _build():
    nc = bacc.Bacc("TRN2", target_bir_lowering=False, debug=False,
                   num_devices=N_CORES)
    xt_d = nc.dram_tensor("xt", [D, BS], dt.float32, kind="ExternalInput").ap()
    w8_d = nc.dram_tensor("w8", [NPAIR * 128, 2 * D], dt.float8e4,
                          kind="ExternalInput").ap()
    w16_d = nc.dram_tensor("w16", [NB16 * 128, D], dt.bfloat16,
                           kind="ExternalInput").ap()
    gb_d = nc.dram_tensor("gb", [128, 2 * KT], dt.float32,
                          kind="ExternalInput").ap()
    out_d = nc.dram_tensor("out", [BS, D], dt.float32,
                           kind="ExternalOutput").ap()

    with tile.TileContext(nc) as tc:
        with (
            tc.tile_pool(name="const", bufs=1) as const,
            tc.tile_pool(name="xs", bufs=3) as xsp,      # phase-1 stream
            tc.tile_pool(name="xr", bufs=NRES) as xrp,   # resident 0..11
            tc.tile_pool(name="xa", bufs=1) as xap,      # 12..15 reload, held
            tc.tile_pool(name="xb", bufs=3) as xbp,      # 16..31 reload
            tc.tile_pool(name="sqd", bufs=1) as sqdp,
            tc.tile_pool(name="habs", bufs=2) as habsp,
            tc.tile_pool(name="hbp", bufs=1) as hbpp,
            tc.tile_pool(name="hbs", bufs=1) as hbsp,
            tc.tile_pool(name="w8", bufs=10) as w8p,
            tc.tile_pool(name="w16", bufs=17) as w16p,
            tc.tile_pool(name="yb", bufs=10) as ybp,
            tc.tile_pool(name="par", bufs=1) as parp,
            tc.tile_pool(name="ps", bufs=7, space="PSUM") as psp,
            tc.tile_pool(name="dram", bufs=1, space="DRAM") as dram,
        ):
            # ---- constants -------------------------------------------------
            gb_t = const.tile([128, 2 * KT], dt.float32)
            nc.scalar.dma_start(gb_t[:], gb_d[:])
            eps_t = const.tile([128, 1], dt.float32)
            nc.vector.memset(eps_t[:], EPS)
            ones = const.tile([128, 1], dt.bfloat16)
            nc.vector.memset(ones[:], 1.0)
            acc = const.tile([128, BS], dt.bfloat16)
            nc.vector.memset(acc[:], 0.0)

            HKS = [16, KT - 16]
            KBASE = [0, 16]
            stat_sum = const.tile([128, KT], dt.float32)
            stat_sq = const.tile([128, KT], dt.float32)
            a_t = const.tile([128, KT], dt.float32)
            b_t = const.tile([128, KT], dt.float32)
            xres = {}

            # ---- phase 1: stream x on two queues + per-shard stats ---------
            def phase1(half):
                for t in range(KBASE[half], KBASE[half] + HKS[half]):
                    if t < NRES:
                        xs = xrp.tile([128, BS], dt.float32, name="xk")
                        xres[t] = xs
                    else:
                        xs = xsp.tile([128, BS], dt.float32, name="xs")
                    q = nc.sync if t % 2 == 0 else nc.gpsimd
                    q.dma_start(xs[:], xt_d[t * 128:(t + 1) * 128, :])
                    nc.vector.reduce_sum(stat_sum[:, t:t + 1], xs[:],
                                         axis=mybir.AxisListType.X)
                    sq = sqdp.tile([128, BS], dt.float8e4, name="sq")
                    nc.scalar.activation(sq[:], xs[:], AF.Square,
                                         accum_out=stat_sq[:, t:t + 1])

            # ---- priming collective: absorbs the ~60us CC-stream bootstrap
            # (barrier + cold first op) in parallel with phase 1, so the real
            # AllReduces run as cheap warm ops (~10us each).
            prime_in = dram.tile([1, 8], dt.float32, name="prime_in",
                                 tag="prime_in")
            prime_out = dram.tile([1, 8], dt.float32, name="prime_out",
                                  tag="prime_out")
            nc.gpsimd.collective_compute(
                "AllReduce", ALU.add,
                replica_groups=[list(range(N_CORES))],
                ins=[prime_in.opt()], outs=[prime_out.opt()],
            )

            # ---- AllReduce staging/trigger, separate from coefficient math -
            outbs = [None, None]

            def ar_stage(half):
                HK = HKS[half]
                ks = slice(KBASE[half], KBASE[half] + HK)
                inb = dram.tile([128, 2 * HK], dt.float32, name=f"inb{half}",
                                tag=f"inb{half}")
                outb = dram.tile([128, 2 * HK], dt.float32,
                                 name=f"outb{half}", tag=f"outb{half}")
                nc.scalar.dma_start(inb[:, 0:HK], stat_sum[:, ks])
                nc.scalar.dma_start(inb[:, HK:2 * HK], stat_sq[:, ks])
                nc.gpsimd.collective_compute(
                    "AllReduce", ALU.add,
                    replica_groups=[list(range(N_CORES))],
                    ins=[inb.opt()], outs=[outb.opt()],
                )
                outbs[half] = outb

            def bn_math(half):
                # a = gamma/std, b = beta - mean*a
                HK = HKS[half]
                k0 = KBASE[half]
                ks = slice(k0, k0 + HK)
                # read-back sits here (not in ar_stage) so its semaphore wait
                # never blocks the phase-1 sq chain on the ACT queue
                st = const.tile([128, 2 * HK], dt.float32,
                                name=f"sall{half}", tag=f"sall{half}")
                nc.scalar.dma_start(st[:], outbs[half][:])
                mean = const.tile([128, HK], dt.float32, name=f"mean{half}",
                                  tag=f"mean{half}")
                nc.vector.tensor_scalar_mul(mean[:], st[:, 0:HK], 1.0 / B)
                var = const.tile([128, HK], dt.float32, name=f"var{half}",
                                 tag=f"var{half}")
                nc.vector.tensor_scalar_mul(var[:], st[:, HK:2 * HK], 1.0 / B)
                msq = const.tile([128, HK], dt.float32, name=f"msq{half}",
                                 tag=f"msq{half}")
                nc.vector.tensor_mul(msq[:], mean[:], mean[:])
                nc.vector.tensor_sub(var[:], var[:], msq[:])
                std = const.tile([128, HK], dt.float32, name=f"std{half}",
                                 tag=f"std{half}")
                nc.scalar.activation(std[:], var[:], AF.Sqrt,
                                     bias=eps_t[:, 0:1], scale=1.0)
                ivs = const.tile([128, HK], dt.float32, name=f"ivs{half}",
                                 tag=f"ivs{half}")
                nc.vector.reciprocal(ivs[:], std[:])
                nc.vector.tensor_mul(a_t[:, ks], ivs[:], gb_t[:, ks])
                nc.vector.tensor_mul(b_t[:, ks], mean[:], a_t[:, ks])
                nc.vector.tensor_sub(
                    b_t[:, ks],
                    gb_t[:, KT + k0:KT + k0 + HK],
                    b_t[:, ks])

            phase1(0)
            ar_stage(0)
            phase1(1)
            ar_stage(1)
            bn_math(0)

            # ---- phase 2a: signs for chunk A (gates the fp8 pre-stage) -----
            hbpair = [hbpp.tile([128, 2 * BS], dt.float8e4, name=f"hbp{i}",
                                tag=f"hbp{i}") for i in range(NPAIR)]
            hbsing = [hbsp.tile([128, BS], dt.float8e4, name=f"hbs{i}",
                                tag=f"hbs{i}") for i in range(NB16)]
            xphase2 = {}
            for t in range(FP8_KT):
                if t in xres:
                    xs = xres[t]
                else:
                    xs = xap.tile([128, BS], dt.float32, name=f"xa{t}",
                                  tag=f"xa{t}", bufs=1)
                    nc.gpsimd.dma_start(xs[:], xt_d[t * 128:(t + 1) * 128, :])
                xphase2[t] = xs
                hb = hbpair[t // 2][:, (t % 2) * BS:(t % 2 + 1) * BS]
                nc.scalar.activation(hb, xs[:], AF.Sign,
                                     bias=b_t[:, t:t + 1],
                                     scale=a_t[:, t:t + 1])

            # ---- phase 3a: fp8 pair pre-stage for oq 0..PRE_OQ-1 -----------
            # Runs under the AR-B / chunk-B sign shadow; partials spill to
            # SBUF in bf16 and are re-added at drain time.
            def mm_pair(pt, w8t, kp, bt, start, stop):
                nc.tensor.matmul(
                    pt[:],
                    hbpair[kp][:].rearrange("p (i b) -> p i b", i=2)
                    [:, :, bt * 128:(bt + 1) * 128],
                    w8t[:].rearrange("p (i o) -> p i o", i=2),
                    start=start, stop=stop, perf_mode=PM.DoubleRow)

            def load_w8(oq):
                w8s = []
                for kp in range(NPAIR):
                    w8t = w8p.tile([128, 2 * 512], dt.float8e4, name="w8t")
                    nc.sync.dma_start(
                        w8t[:].rearrange("p (i o) -> p i o", i=2),
                        w8_d[kp * 128:(kp + 1) * 128, :]
                        .rearrange("p (i o) -> p i o", i=2)
                        [:, :, oq * 512:(oq + 1) * 512])
                    w8s.append(w8t)
                return w8s

            def load_w16(oq):
                w16s = []
                for j in range(NB16):
                    wtt = w16p.tile([128, 512], dt.bfloat16, name="wtt")
                    nc.sync.dma_start(
                        wtt[:],
                        w16_d[j * 128:(j + 1) * 128,
                              oq * 512:(oq + 1) * 512])
                    w16s.append(wtt)
                return w16s

            # Per-bt cyclic rotation of the k order spreads each W tile's
            # last-read across the oq span, so the next oq's W prefetch
            # trickles in instead of bursting at the boundary (which left
            # the PE idle long enough for HAM to re-throttle).
            partials = {}
            spills = []
            for oq in range(PRE_OQ):
                w8s = load_w8(oq)
                for bt in range(NBT):
                    pt = psp.tile([128, 512], dt.float32, name="pt", tag="pt")
                    for i in range(NPAIR):
                        kp = (bt + i) % NPAIR
                        mm_pair(pt, w8s[kp], kp, bt, i == 0, i == NPAIR - 1)
                    par = parp.tile([128, 512], dt.bfloat16,
                                    name=f"par{oq}_{bt}",
                                    tag=f"par{oq}_{bt}", bufs=1)
                    partials[(oq, bt)] = par
                    spills.append((par, pt))

            # DVE: spills for oq0 first (they recycle PSUM banks for the
            # pre-stage); the rest interleave with chunk-B |h| accumulation.
            for par, pt in spills[:NBT]:
                nc.vector.tensor_copy(par[:], pt[:])
            rest = list(spills[NBT:])

            bn_math(1)

            # w16 for oq0 must land ahead of the phase-2b x refetches that
            # share the sync queue (the main loop needs it right away).
            w16_oq0 = load_w16(0)

            # ---- phase 2b: chunk-B signs + all |h| accumulation ------------
            # ACT order: sign16, abs16, sign17, abs17, ... then abs for chunk
            # A at the end (resident/held tiles -- no buffer pressure).
            def acc_abs(t, xs):
                habs = habsp.tile([128, BS], dt.bfloat16, name="habs",
                                  tag="habs")
                nc.scalar.activation(habs[:], xs[:], AF.Abs,
                                     bias=b_t[:, t:t + 1],
                                     scale=a_t[:, t:t + 1])
                nc.vector.tensor_add(acc[:], acc[:], habs[:])

            for t in range(FP8_KT, KT):
                xs = xbp.tile([128, BS], dt.float32, name="xb")
                nc.sync.dma_start(xs[:], xt_d[t * 128:(t + 1) * 128, :])
                nc.scalar.activation(hbsing[t - FP8_KT][:], xs[:], AF.Sign,
                                     bias=b_t[:, t:t + 1],
                                     scale=a_t[:, t:t + 1])
                acc_abs(t, xs)
                # interleave ~1.5 pre-stage spills per chunk-B tile on DVE
                for _ in range(2 if t % 2 == 0 else 1):
                    if rest:
                        par, pt = rest.pop(0)
                        nc.vector.tensor_copy(par[:], pt[:])
            for par, pt in rest:
                nc.vector.tensor_copy(par[:], pt[:])
            for t in range(FP8_KT):
                acc_abs(t, xphase2[t])

            # ---- main matmul stream ---------------------------------------
            # oq < PRE_OQ: bf16 k-tiles only + partial re-add; oq >= PRE_OQ:
            # full 8-pair + 16-single accumulation.  Beta's two ones-matmuls
            # slot in after oq0 (acc is complete by then); oq0's drains are
            # deferred-scaled once betaT lands.
            beta_ready = False
            betaT = None
            deferred = []
            for oq in range(NOQ):
                w16s = w16_oq0 if oq == 0 else load_w16(oq)
                w8s = None if oq < PRE_OQ else load_w8(oq)
                for bt in range(NBT):
                    pt = psp.tile([128, 512], dt.float32, name="pt", tag="pt")
                    if oq < PRE_OQ:
                        for i in range(NB16):
                            j = (2 * bt + i) % NB16
                            nc.tensor.matmul(
                                pt[:], hbsing[j][:, bt * 128:(bt + 1) * 128],
                                w16s[j][:], start=(i == 0),
                                stop=(i == NB16 - 1))
                    else:
                        for i in range(NPAIR):
                            kp = (bt + i) % NPAIR
                            mm_pair(pt, w8s[kp], kp, bt, i == 0, False)
                        for i in range(NB16):
                            j = (2 * bt + i) % NB16
                            nc.tensor.matmul(
                                pt[:], hbsing[j][:, bt * 128:(bt + 1) * 128],
                                w16s[j][:], start=False,
                                stop=(i == NB16 - 1))
                    if not beta_ready:
                        # beta not ready yet: drain unscaled, scale later
                        ybu = ybp.tile([128, 512], dt.float32, name="ybu",
                                       tag="yb")
                        nc.vector.tensor_add(ybu[:], pt[:],
                                             partials[(oq, bt)][:])
                        deferred.append((oq, bt, ybu))
                    else:
                        yb = ybp.tile([128, 512], dt.float32, name="yb",
                                      tag="yb")
                        if oq < PRE_OQ:
                            tmp = ybp.tile([128, 512], dt.float32, name="tmp",
                                           tag="yb")
                            nc.vector.tensor_add(tmp[:], pt[:],
                                                 partials[(oq, bt)][:])
                            nc.vector.tensor_scalar_mul(
                                yb[:], tmp[:], betaT[:, bt:bt + 1])
                        else:
                            nc.vector.tensor_scalar_mul(
                                yb[:], pt[:], betaT[:, bt:bt + 1])
                        nc.gpsimd.dma_start(
                            out_d[bt * 128:(bt + 1) * 128,
                                  oq * 512:(oq + 1) * 512], yb[:])

                if oq == 0:
                    # ---- beta: ones-matmul over the |h| accumulator --------
                    # the two 512-col halves sit on partitions 0 and 32 of
                    # ONE psum bank ([1, 1024] would span two banks)
                    beta_ps = psp.tile([33, 512], dt.float32, tag="beta",
                                       bufs=1)
                    for half in range(BS // 512):
                        nc.tensor.matmul(
                            beta_ps[half * 32:half * 32 + 1, :], ones[:],
                            acc[:, half * 512:(half + 1) * 512],
                            start=True, stop=True)
                    beta_sb = const.tile([33, 512], dt.float32)
                    nc.vector.tensor_scalar_mul(beta_sb[:], beta_ps[:],
                                                1.0 / (D * W_SCALE))
                    # [1, BS] -> [128, BS/128] transpose via a DRAM bounce
                    # (DRAM-side access patterns are unconstrained)
                    bb = dram.tile([1, BS], dt.float32)
                    nc.scalar.dma_start(bb[:, 0:512], beta_sb[0:1, :])
                    nc.scalar.dma_start(bb[:, 512:BS], beta_sb[32:33, :])
                    betaT = const.tile([128, BS // 128], dt.float32)
                    nc.scalar.dma_start(
                        betaT[:], bb.rearrange("o (j p) -> (o p) j", p=128))
                    beta_ready = True
                    # late scale + writeback for the deferred oq0 drains
                    for doq, dbt, ybu in deferred:
                        yb = ybp.tile([128, 512], dt.float32, name="yb2",
                                      tag="yb")
                        nc.vector.tensor_scalar_mul(
                            yb[:], ybu[:], betaT[:, dbt:dbt + 1])
                        nc.gpsimd.dma_start(
                            out_d[dbt * 128:(dbt + 1) * 128,
                                  doq * 512:(doq + 1) * 512], yb[:])

    nc.compile()
    return nc


def kernel(x, bn_gamma, bn_beta, W, alpha):
    global _nc_cache, LAST_RESULT
    x = np.ascontiguousarray(x, dtype=np.float32)
    W = np.asarray(W, dtype=np.float32)
    alpha = np.asarray(alpha, dtype=np.float32)

    # host prep: fold alpha into W, transpose to [in, out], upscale by 2^12
    ws = np.ascontiguousarray((W * alpha[:, None]).T) * np.float32(W_SCALE)
    # fp8 part: k-tiles 0..15 -> pair layout [kp*128+p, i*D+o],
    # value = ws[(2kp + i)*128 + p, o]
    w8 = ws[:FP8_KT * 128].reshape(NPAIR, 2, 128, D).transpose(0, 2, 1, 3)
    w8 = np.clip(w8, -240.0, 240.0).reshape(NPAIR * 128, 2 * D)
    w8 = np.ascontiguousarray(w8).astype(ml_dtypes.float8_e4m3)
    # bf16 part: k-tiles 16..31 (same 2^12 scale -- exact in bf16)
    w16 = np.ascontiguousarray(ws[FP8_KT * 128:]).astype(ml_dtypes.bfloat16)
    # gamma/beta in per-partition layout: gb[p, t] = gamma[t*128 + p]
    gb = np.concatenate(
        [np.asarray(bn_gamma, np.float32).reshape(KT, 128).T,
         np.asarray(bn_beta, np.float32).reshape(KT, 128).T], axis=1)
    gb = np.ascontiguousarray(gb)

    if _nc_cache is None:
        _nc_cache = _build()
    nc = _nc_cache

    in_maps = []
    for c in range(N_CORES):
        xT = np.ascontiguousarray(x[c * BS:(c + 1) * BS, :].T)
        in_maps.append({"xt": xT, "w8": w8, "w16": w16, "gb": gb})

    res = run_bass_kernel_spmd(nc, in_maps, core_ids=list(range(N_CORES)),
                               trace=TRACE)
    LAST_RESULT = res
    return np.concatenate([res.results[c]["out"] for c in range(N_CORES)],
                          axis=0)


# revision 13
# speedup vs baseline: 1.0805x; 1.0805x over previous
"""BinLinear (BatchNorm -> sign-binarize -> scaled binary linear) on 8 TRN2
NeuronCores.

Reference computation (fp32, full batch):
    mean/var over batch axis of x [8192, 4096]
    h  = (x - mean) * rsqrt(var + eps) * gamma + beta          [8192, 4096]
    hb = sign(h)
    out = (hb @ W.T) * alpha[None, :] * mean_abs(h, axis=1)    [8192, 4096]

Distribution: data-parallel over the batch (1024 rows per core); a ~64 KB
AllReduce (two pipelined 32 KB chunks) produces full-batch BN statistics.

Device-side structure (per core, xT layout [feat, batch] so features sit on
SBUF partitions).  The critical path is AR-A -> signs -> matmul; everything
else (|h|, beta, BN-B coefficients) hides under it:
  phase 1  x streams in on TWO DMA queues (sync: even k-tiles, gpsimd: odd);
           per-feature sum on DVE, sum-of-squares on ACT.  Emitted in two
           halves so the chunk-A AllReduce staging + trigger land on their
           queues ahead of the second half's loads.  Tiles 0..11 stay
           resident; the rest are re-fetched for phase 2.
  AllReduce chunk A = k-tiles 0..15 (all the fp8 pair tiles) fires as soon
           as their stats land; chunk B = 16..31 follows on the CC stream.
           The BN coefficient math is emitted separately from the staging so
           the ACT Sqrt never blocks phase-1 stats or the sign chains.
  phase 2  hb = Sign(a*x + b) on ACT (fp8, exact +-1).  |h| = Abs(a*x+b) on
           ACT strictly after the signs it would delay; per-feature |h|
           accumulates across k-tiles on DVE (bf16) and one ones-matmul pair
           at the end gives the per-row beta -- no per-tile beta matmuls.
  phase 3  out = hb.T @ W, k-tiles 0..15 as fp8 DoubleRow pairs (K=256 per
           instruction), 16..31 in bf16.  While AR-B / chunk-B signs are in
           flight, a pair-only pre-stage runs the 8 DoubleRow matmuls for
           oq0..3 x all bt and spills the partials to SBUF in bf16; the main
           stream re-adds them and runs only the bf16 k-tiles for those oq.
           W is pre-scaled by alpha and 2^12 on the host (fp8 subnormal
           floor); the 2^-12 is folded into the beta scale.  Output tiles
           that complete before beta does are drained unscaled and scaled
           once beta lands.
"""

import numpy as np
import ml_dtypes

import concourse.bacc as bacc
import concourse.mybir as mybir
import concourse.tile as tile
from concourse.bass_utils import run_bass_kernel_spmd

dt = mybir.dt
AF = mybir.ActivationFunctionType
ALU = mybir.AluOpType
PM = mybir.MatmulPerfMode

N_CORES = 8
B, D = 8192, 4096          # batch, features (D_in == D_out == 4096)
BS = B // N_CORES          # 1024 batch rows per core
KT = D // 128              # 32 k-tiles (features / partitions)
EPS = 1e-5

FP8_KT = 16                # k-tiles 0..15 go through fp8 DoubleRow pairs
NPAIR = FP8_KT // 2        # 8 pair tiles
NB16 = KT - FP8_KT         # 16 bf16 k-tiles
W_SCALE = 4096.0           # 2^12 upscale for fp8 W (folded out via beta)

NRES = 12                  # k-tiles 0..11 stay resident in SBUF
PRE_OQ = 3                 # oq 0..2 get the pair-only pre-stage
NBT = BS // 128            # 8 batch tiles
NOQ = D // 512             # 8 output quarters

TRACE = False              # set by test.py for profiling runs
LAST_RESULT = None

_nc_cache = None


def _build():
    nc = bacc.Bacc("TRN2", target_bir_lowering=False, debug=False,
                   num_devices=N_CORES)
    xt_d = nc.dram_tensor("xt", [D, BS], dt.float32, kind="ExternalInput").ap()
    w8_d = nc.dram_tensor("w8", [NPAIR * 128, 2 * D], dt.float8e4,
                          kind="ExternalInput").ap()
    w16_d = nc.dram_tensor("w16", [NB16 * 128, D], dt.bfloat16,
                           kind="ExternalInput").ap()
    gb_d = nc.dram_tensor("gb", [128, 2 * KT], dt.float32,
                          kind="ExternalInput").ap()
    out_d = nc.dram_tensor("out", [BS, D], dt.float32,
                           kind="ExternalOutput").ap()

    with tile.TileContext(nc) as tc:
        with (
            tc.tile_pool(name="const", bufs=1) as const,
            tc.tile_pool(name="xs", bufs=3) as xsp,      # phase-1 stream
            tc.tile_pool(name="xr", bufs=NRES) as xrp,   # resident 0..11
            tc.tile_pool(name="xa", bufs=1) as xap,      # 12..15 reload, held
            tc.tile_pool(name="xb", bufs=3) as xbp,      # 16..31 reload
            tc.tile_pool(name="sqd", bufs=1) as sqdp,
            tc.tile_pool(name="habs", bufs=2) as habsp,
            tc.tile_pool(name="hbp", bufs=1) as hbpp,
            tc.tile_pool(name="hbs", bufs=1) as hbsp,
            tc.tile_pool(name="w8", bufs=10) as w8p,
            tc.tile_pool(name="w16", bufs=17) as w16p,
            tc.tile_pool(name="yb", bufs=10) as ybp,
            tc.tile_pool(name="par", bufs=1) as parp,
            tc.tile_pool(name="ps", bufs=7, space="PSUM") as psp,
            tc.tile_pool(name="dram", bufs=1, space="DRAM") as dram,
        ):
            # ---- constants -------------------------------------------------
            gb_t = const.tile([128, 2 * KT], dt.float32)
            nc.scalar.dma_start(gb_t[:], gb_d[:])
            eps_t = const.tile([128, 1], dt.float32)
            nc.vector.memset(eps_t[:], EPS)
            ones = const.tile([128, 1], dt.bfloat16)
            nc.vector.memset(ones[:], 1.0)
            acc = const.tile([128, BS], dt.bfloat16)
            nc.vector.memset(acc[:], 0.0)

            HKS = [16, KT - 16]
            KBASE = [0, 16]
            stat_sum = const.tile([128, KT], dt.float32)
            stat_sq = const.tile([128, KT], dt.float32)
            a_t = const.tile([128, KT], dt.float32)
            b_t = const.tile([128, KT], dt.float32)
            xres = {}

            # ---- phase 1: stream x on two queues + per-shard stats ---------
            def phase1(half):
                for t in range(KBASE[half], KBASE[half] + HKS[half]):
                    if t < NRES:
                        xs = xrp.tile([128, BS], dt.float32, name="xk")
                        xres[t] = xs
                    else:
                        xs = xsp.tile([128, BS], dt.float32, name="xs")
                    q = nc.sync if t % 2 == 0 else nc.scalar
                    q.dma_start(xs[:], xt_d[t * 128:(t + 1) * 128, :])
                    nc.vector.reduce_sum(stat_sum[:, t:t + 1], xs[:],
                                         axis=mybir.AxisListType.X)
                    sq = sqdp.tile([128, BS], dt.float8e4, name="sq")
                    nc.scalar.activation(sq[:], xs[:], AF.Square,
                                         accum_out=stat_sq[:, t:t + 1])

            # ---- AllReduce staging/trigger, separate from coefficient math -
            # NOTE: a collective enqueued before the phase-1 DMAs stalls their
            # completions until the CC barrier clears (measured +40us), so no
            # priming op, and the gpsimd queue carries ONLY the two triggers
            # (a trigger blocks its queue until the collective finishes)
            # followed by the output drains.
            outbs = [None, None]

            def ar_stage(half):
                HK = HKS[half]
                ks = slice(KBASE[half], KBASE[half] + HK)
                inb = dram.tile([128, 2 * HK], dt.float32, name=f"inb{half}",
                                tag=f"inb{half}")
                outb = dram.tile([128, 2 * HK], dt.float32,
                                 name=f"outb{half}", tag=f"outb{half}")
                nc.scalar.dma_start(inb[:, 0:HK], stat_sum[:, ks])
                nc.scalar.dma_start(inb[:, HK:2 * HK], stat_sq[:, ks])
                nc.gpsimd.collective_compute(
                    "AllReduce", ALU.add,
                    replica_groups=[list(range(N_CORES))],
                    ins=[inb.opt()], outs=[outb.opt()],
                )
                outbs[half] = outb

            def bn_math(half):
                # a = gamma/std, b = beta - mean*a
                HK = HKS[half]
                k0 = KBASE[half]
                ks = slice(k0, k0 + HK)
                # read-back sits here (not in ar_stage) so its semaphore wait
                # never blocks the phase-1 sq chain on the ACT queue
                st = const.tile([128, 2 * HK], dt.float32,
                                name=f"sall{half}", tag=f"sall{half}")
                nc.scalar.dma_start(st[:], outbs[half][:])
                mean = const.tile([128, HK], dt.float32, name=f"mean{half}",
                                  tag=f"mean{half}")
                nc.vector.tensor_scalar_mul(mean[:], st[:, 0:HK], 1.0 / B)
                var = const.tile([128, HK], dt.float32, name=f"var{half}",
                                 tag=f"var{half}")
                nc.vector.tensor_scalar_mul(var[:], st[:, HK:2 * HK], 1.0 / B)
                msq = const.tile([128, HK], dt.float32, name=f"msq{half}",
                                 tag=f"msq{half}")
                nc.vector.tensor_mul(msq[:], mean[:], mean[:])
                nc.vector.tensor_sub(var[:], var[:], msq[:])
                std = const.tile([128, HK], dt.float32, name=f"std{half}",
                                 tag=f"std{half}")
                nc.scalar.activation(std[:], var[:], AF.Sqrt,
                                     bias=eps_t[:, 0:1], scale=1.0)
                ivs = const.tile([128, HK], dt.float32, name=f"ivs{half}",
                                 tag=f"ivs{half}")
                nc.vector.reciprocal(ivs[:], std[:])
                nc.vector.tensor_mul(a_t[:, ks], ivs[:], gb_t[:, ks])
                nc.vector.tensor_mul(b_t[:, ks], mean[:], a_t[:, ks])
                nc.vector.tensor_sub(
                    b_t[:, ks],
                    gb_t[:, KT + k0:KT + k0 + HK],
                    b_t[:, ks])

            phase1(0)
            ar_stage(0)
            phase1(1)

            # early reloads for the held chunk-A tail (12..15): issued on the
            # ACT queue before the AR read-backs so they never block behind
            # the collective's completion wait
            xphase2 = {}
            for t in range(NRES, FP8_KT):
                xs = xap.tile([128, BS], dt.float32, name=f"xa{t}",
                              tag=f"xa{t}", bufs=1)
                nc.scalar.dma_start(xs[:], xt_d[t * 128:(t + 1) * 128, :])
                xphase2[t] = xs

            ar_stage(1)
            bn_math(0)

            # ---- phase 2a: signs for chunk A (gates the fp8 pre-stage) -----
            hbpair = [hbpp.tile([128, 2 * BS], dt.float8e4, name=f"hbp{i}",
                                tag=f"hbp{i}") for i in range(NPAIR)]
            hbsing = [hbsp.tile([128, BS], dt.float8e4, name=f"hbs{i}",
                                tag=f"hbs{i}") for i in range(NB16)]
            for t in range(FP8_KT):
                xs = xres[t] if t in xres else xphase2[t]
                xphase2[t] = xs
                hb = hbpair[t // 2][:, (t % 2) * BS:(t % 2 + 1) * BS]
                nc.scalar.activation(hb, xs[:], AF.Sign,
                                     bias=b_t[:, t:t + 1],
                                     scale=a_t[:, t:t + 1])

            # ---- phase 3a: fp8 pair pre-stage for oq 0..PRE_OQ-1 -----------
            # Runs under the AR-B / chunk-B sign shadow; partials spill to
            # SBUF in bf16 and are re-added at drain time.
            def mm_pair(pt, w8t, kp, bt, start, stop):
                nc.tensor.matmul(
                    pt[:],
                    hbpair[kp][:].rearrange("p (i b) -> p i b", i=2)
                    [:, :, bt * 128:(bt + 1) * 128],
                    w8t[:].rearrange("p (i o) -> p i o", i=2),
                    start=start, stop=stop, perf_mode=PM.DoubleRow)

            def load_w8(oq):
                w8s = []
                for kp in range(NPAIR):
                    w8t = w8p.tile([128, 2 * 512], dt.float8e4, name="w8t")
                    nc.sync.dma_start(
                        w8t[:].rearrange("p (i o) -> p i o", i=2),
                        w8_d[kp * 128:(kp + 1) * 128, :]
                        .rearrange("p (i o) -> p i o", i=2)
                        [:, :, oq * 512:(oq + 1) * 512])
                    w8s.append(w8t)
                return w8s

            def load_w16(oq):
                w16s = []
                for j in range(NB16):
                    wtt = w16p.tile([128, 512], dt.bfloat16, name="wtt")
                    nc.sync.dma_start(
                        wtt[:],
                        w16_d[j * 128:(j + 1) * 128,
                              oq * 512:(oq + 1) * 512])
                    w16s.append(wtt)
                return w16s

            # Per-bt cyclic rotation of the k order spreads each W tile's
            # last-read across the oq span, so the next oq's W prefetch
            # trickles in instead of bursting at the boundary (which left
            # the PE idle long enough for HAM to re-throttle).
            partials = {}
            spills = []
            for oq in range(PRE_OQ):
                w8s = load_w8(oq)
                for bt in range(NBT):
                    pt = psp.tile([128, 512], dt.float32, name="pt", tag="pt")
                    for i in range(NPAIR):
                        kp = (bt + i) % NPAIR
                        mm_pair(pt, w8s[kp], kp, bt, i == 0, i == NPAIR - 1)
                    par = parp.tile([128, 512], dt.bfloat16,
                                    name=f"par{oq}_{bt}",
                                    tag=f"par{oq}_{bt}", bufs=1)
                    partials[(oq, bt)] = par
                    spills.append((par, pt))

            # DVE: spills for oq0 first (they recycle PSUM banks for the
            # pre-stage); the rest interleave with chunk-B |h| accumulation.
            for par, pt in spills[:NBT]:
                nc.vector.tensor_copy(par[:], pt[:])
            rest = list(spills[NBT:])

            bn_math(1)

            # w16 for oq0 must land ahead of the phase-2b x refetches that
            # share the sync queue (the main loop needs it right away).
            w16_oq0 = load_w16(0)

            # ---- phase 2b: chunk-B signs + all |h| accumulation ------------
            # ACT order: sign16, abs16, sign17, abs17, ... then abs for chunk
            # A at the end (resident/held tiles -- no buffer pressure).
            def acc_abs(t, xs):
                habs = habsp.tile([128, BS], dt.bfloat16, name="habs",
                                  tag="habs")
                nc.scalar.activation(habs[:], xs[:], AF.Abs,
                                     bias=b_t[:, t:t + 1],
                                     scale=a_t[:, t:t + 1])
                nc.vector.tensor_add(acc[:], acc[:], habs[:])

            for t in range(FP8_KT, KT):
                xs = xbp.tile([128, BS], dt.float32, name="xb")
                nc.sync.dma_start(xs[:], xt_d[t * 128:(t + 1) * 128, :])
                nc.scalar.activation(hbsing[t - FP8_KT][:], xs[:], AF.Sign,
                                     bias=b_t[:, t:t + 1],
                                     scale=a_t[:, t:t + 1])
                acc_abs(t, xs)
                # interleave ~1.5 pre-stage spills per chunk-B tile on DVE
                for _ in range(2 if t % 2 == 0 else 1):
                    if rest:
                        par, pt = rest.pop(0)
                        nc.vector.tensor_copy(par[:], pt[:])
            for par, pt in rest:
                nc.vector.tensor_copy(par[:], pt[:])
            for t in range(FP8_KT):
                acc_abs(t, xphase2[t])

            # ---- main matmul stream ---------------------------------------
            # oq < PRE_OQ: bf16 k-tiles only + partial re-add; oq >= PRE_OQ:
            # full 8-pair + 16-single accumulation.  Beta's two ones-matmuls
            # slot in after oq0 (acc is complete by then); oq0's drains are
            # deferred-scaled once betaT lands.
            beta_ready = False
            betaT = None
            deferred = []
            for oq in range(NOQ):
                w16s = w16_oq0 if oq == 0 else load_w16(oq)
                w8s = None if oq < PRE_OQ else load_w8(oq)
                for bt in range(NBT):
                    pt = psp.tile([128, 512], dt.float32, name="pt", tag="pt")
                    if oq < PRE_OQ:
                        for i in range(NB16):
                            j = (2 * bt + i) % NB16
                            nc.tensor.matmul(
                                pt[:], hbsing[j][:, bt * 128:(bt + 1) * 128],
                                w16s[j][:], start=(i == 0),
                                stop=(i == NB16 - 1))
                    else:
                        for i in range(NPAIR):
                            kp = (bt + i) % NPAIR
                            mm_pair(pt, w8s[kp], kp, bt, i == 0, False)
                        for i in range(NB16):
                            j = (2 * bt + i) % NB16
                            nc.tensor.matmul(
                                pt[:], hbsing[j][:, bt * 128:(bt + 1) * 128],
                                w16s[j][:], start=False,
                                stop=(i == NB16 - 1))
                    if not beta_ready:
                        # beta not ready yet: drain unscaled, scale later
                        ybu = ybp.tile([128, 512], dt.float32, name="ybu",
                                       tag="yb")
                        nc.vector.tensor_add(ybu[:], pt[:],
                                             partials[(oq, bt)][:])
                        deferred.append((oq, bt, ybu))
                    else:
                        yb = ybp.tile([128, 512], dt.float32, name="yb",
                                      tag="yb")
                        if oq < PRE_OQ:
                            tmp = ybp.tile([128, 512], dt.float32, name="tmp",
                                           tag="yb")
                            nc.vector.tensor_add(tmp[:], pt[:],
                                                 partials[(oq, bt)][:])
                            nc.vector.tensor_scalar_mul(
                                yb[:], tmp[:], betaT[:, bt:bt + 1])
                        else:
                            nc.vector.tensor_scalar_mul(
                                yb[:], pt[:], betaT[:, bt:bt + 1])
                        nc.gpsimd.dma_start(
                            out_d[bt * 128:(bt + 1) * 128,
                                  oq * 512:(oq + 1) * 512], yb[:])

                if oq == 0:
                    # ---- beta: ones-matmul over the |h| accumulator --------
                    # the two 512-col halves sit on partitions 0 and 32 of
                    # ONE psum bank ([1, 1024] would span two banks)
                    beta_ps = psp.tile([33, 512], dt.float32, tag="beta",
                                       bufs=1)
                    for half in range(BS // 512):
                        nc.tensor.matmul(
                            beta_ps[half * 32:half * 32 + 1, :], ones[:],
                            acc[:, half * 512:(half + 1) * 512],
                            start=True, stop=True)
                    beta_sb = const.tile([33, 512], dt.float32)
                    nc.vector.tensor_scalar_mul(beta_sb[:], beta_ps[:],
                                                1.0 / (D * W_SCALE))
                    # [1, BS] -> [128, BS/128] transpose via a DRAM bounce
                    # (DRAM-side access patterns are unconstrained)
                    bb = dram.tile([1, BS], dt.float32)
                    nc.scalar.dma_start(bb[:, 0:512], beta_sb[0:1, :])
                    nc.scalar.dma_start(bb[:, 512:BS], beta_sb[32:33, :])
                    betaT = const.tile([128, BS // 128], dt.float32)
                    nc.scalar.dma_start(
                        betaT[:], bb.rearrange("o (j p) -> (o p) j", p=128))
                    beta_ready = True
                    # late scale + writeback for the deferred oq0 drains
                    for doq, dbt, ybu in deferred:
                        yb = ybp.tile([128, 512], dt.float32, name="yb2",
                                      tag="yb")
                        nc.vector.tensor_scalar_mul(
                            yb[:], ybu[:], betaT[:, dbt:dbt + 1])
                        nc.gpsimd.dma_start(
                            out_d[dbt * 128:(dbt + 1) * 128,
                                  doq * 512:(doq + 1) * 512], yb[:])

    nc.compile()
    return nc


def kernel(x, bn_gamma, bn_beta, W, alpha):
    global _nc_cache, LAST_RESULT
    x = np.ascontiguousarray(x, dtype=np.float32)
    W = np.asarray(W, dtype=np.float32)
    alpha = np.asarray(alpha, dtype=np.float32)

    # host prep: fold alpha into W, transpose to [in, out], upscale by 2^12
    ws = np.ascontiguousarray((W * alpha[:, None]).T) * np.float32(W_SCALE)
    # fp8 part: k-tiles 0..15 -> pair layout [kp*128+p, i*D+o],
    # value = ws[(2kp + i)*128 + p, o]
    w8 = ws[:FP8_KT * 128].reshape(NPAIR, 2, 128, D).transpose(0, 2, 1, 3)
    w8 = np.clip(w8, -240.0, 240.0).reshape(NPAIR * 128, 2 * D)
    w8 = np.ascontiguousarray(w8).astype(ml_dtypes.float8_e4m3)
    # bf16 part: k-tiles 16..31 (same 2^12 scale -- exact in bf16)
    w16 = np.ascontiguousarray(ws[FP8_KT * 128:]).astype(ml_dtypes.bfloat16)
    # gamma/beta in per-partition layout: gb[p, t] = gamma[t*128 + p]
    gb = np.concatenate(
        [np.asarray(bn_gamma, np.float32).reshape(KT, 128).T,
         np.asarray(bn_beta, np.float32).reshape(KT, 128).T], axis=1)
    gb = np.ascontiguousarray(gb)

    if _nc_cache is None:
        _nc_cache = _build()
    nc = _nc_cache

    in_maps = []
    for c in range(N_CORES):
        xT = np.ascontiguousarray(x[c * BS:(c + 1) * BS, :].T)
        in_maps.append({"xt": xT, "w8": w8, "w16": w16, "gb": gb})

    res = run_bass_kernel_spmd(nc, in_maps, core_ids=list(range(N_CORES)),
                               trace=TRACE)
    LAST_RESULT = res
    return np.concatenate([res.results[c]["out"] for c in range(N_CORES)],
                          axis=0)


# revision 17
# speedup vs baseline: 1.1939x; 1.1050x over previous
"""BinLinear (BatchNorm -> sign-binarize -> scaled binary linear) on 8 TRN2
NeuronCores.

Reference computation (fp32, full batch):
    mean/var over batch axis of x [8192, 4096]
    h  = (x - mean) * rsqrt(var + eps) * gamma + beta          [8192, 4096]
    hb = sign(h)
    out = (hb @ W.T) * alpha[None, :] * mean_abs(h, axis=1)    [8192, 4096]

Distribution: data-parallel over the batch (1024 rows per core); a ~64 KB
AllReduce (two pipelined 32 KB chunks) produces full-batch BN statistics.

Device-side structure (per core, xT layout [feat, batch] so features sit on
SBUF partitions).  The critical path is AR-A -> signs -> matmul; everything
else (|h|, beta, BN-B coefficients) hides under it:
  phase 1  x streams in on TWO DMA queues (sync: even k-tiles, gpsimd: odd);
           per-feature sum on DVE, sum-of-squares on ACT.  Emitted in two
           halves so the chunk-A AllReduce staging + trigger land on their
           queues ahead of the second half's loads.  Tiles 0..11 stay
           resident; the rest are re-fetched for phase 2.
  AllReduce chunk A = k-tiles 0..15 (all the fp8 pair tiles) fires as soon
           as their stats land; chunk B = 16..31 follows on the CC stream.
           The BN coefficient math is emitted separately from the staging so
           the ACT Sqrt never blocks phase-1 stats or the sign chains.
  phase 2  hb = Sign(a*x + b) on ACT (fp8, exact +-1).  |h| = Abs(a*x+b) on
           ACT strictly after the signs it would delay; per-feature |h|
           accumulates across k-tiles on DVE (bf16) and one ones-matmul pair
           at the end gives the per-row beta -- no per-tile beta matmuls.
  phase 3  out = hb.T @ W, k-tiles 0..15 as fp8 DoubleRow pairs (K=256 per
           instruction), 16..31 in bf16.  While AR-B / chunk-B signs are in
           flight, a pair-only pre-stage runs the 8 DoubleRow matmuls for
           oq0..3 x all bt and spills the partials to SBUF in bf16; the main
           stream re-adds them and runs only the bf16 k-tiles for those oq.
           W is pre-scaled by alpha and 2^12 on the host (fp8 subnormal
           floor); the 2^-12 is folded into the beta scale.  Output tiles
           that complete before beta does are drained unscaled and scaled
           once beta lands.
"""

import numpy as np
import ml_dtypes

import concourse.bacc as bacc
import concourse.mybir as mybir
import concourse.tile as tile
from concourse.bass_utils import run_bass_kernel_spmd

dt = mybir.dt
AF = mybir.ActivationFunctionType
ALU = mybir.AluOpType
PM = mybir.MatmulPerfMode

N_CORES = 8
B, D = 8192, 4096          # batch, features (D_in == D_out == 4096)
BS = B // N_CORES          # 1024 batch rows per core
KT = D // 128              # 32 k-tiles (features / partitions)
EPS = 1e-5

FP8_KT = 16                # k-tiles 0..15 go through fp8 DoubleRow pairs
NPAIR = FP8_KT // 2        # 8 pair tiles
NB16 = KT - FP8_KT         # 16 bf16 k-tiles
W_SCALE = 4096.0           # 2^12 upscale for fp8 W (folded out via beta)

NRES = 12                  # k-tiles 0..11 stay resident in SBUF
PRE_OQ = 3                 # oq 0..2 get the pair-only pre-stage
NBT = BS // 128            # 8 batch tiles
NOQ = D // 512             # 8 output quarters

TRACE = False              # set by test.py for profiling runs
LAST_RESULT = None

_nc_cache = None


def _build():
    nc = bacc.Bacc("TRN2", target_bir_lowering=False, debug=False,
                   num_devices=N_CORES)
    xt_d = nc.dram_tensor("xt", [D, BS], dt.float32, kind="ExternalInput").ap()
    w8_d = nc.dram_tensor("w8", [NPAIR * 128, 2 * D], dt.float8e4,
                          kind="ExternalInput").ap()
    w16_d = nc.dram_tensor("w16", [NB16 * 128, D], dt.bfloat16,
                           kind="ExternalInput").ap()
    gb_d = nc.dram_tensor("gb", [128, 2 * KT], dt.float32,
                          kind="ExternalInput").ap()
    out_d = nc.dram_tensor("out", [BS, D], dt.float32,
                           kind="ExternalOutput").ap()

    with tile.TileContext(nc) as tc:
        with (
            tc.tile_pool(name="const", bufs=1) as const,
            tc.tile_pool(name="xs", bufs=3) as xsp,      # phase-1 stream
            tc.tile_pool(name="xr", bufs=NRES) as xrp,   # resident 0..11
            tc.tile_pool(name="xa", bufs=1) as xap,      # 12..15 reload, held
            tc.tile_pool(name="xb", bufs=3) as xbp,      # 16..31 reload
            tc.tile_pool(name="sqd", bufs=1) as sqdp,
            tc.tile_pool(name="habs", bufs=2) as habsp,
            tc.tile_pool(name="hbp", bufs=1) as hbpp,
            tc.tile_pool(name="hbs", bufs=1) as hbsp,
            tc.tile_pool(name="w8", bufs=10) as w8p,
            tc.tile_pool(name="w16", bufs=17) as w16p,
            tc.tile_pool(name="yb", bufs=10) as ybp,
            tc.tile_pool(name="par", bufs=1) as parp,
            tc.tile_pool(name="ps", bufs=7, space="PSUM") as psp,
            tc.tile_pool(name="dram", bufs=1, space="DRAM") as dram,
        ):
            # ---- constants -------------------------------------------------
            gb_t = const.tile([128, 2 * KT], dt.float32)
            nc.scalar.dma_start(gb_t[:], gb_d[:])
            eps_t = const.tile([128, 1], dt.float32)
            nc.vector.memset(eps_t[:], EPS)
            ones = const.tile([128, 1], dt.bfloat16)
            nc.vector.memset(ones[:], 1.0)
            acc = const.tile([128, BS], dt.bfloat16)
            nc.vector.memset(acc[:], 0.0)

            HKS = [16, KT - 16]
            KBASE = [0, 16]
            stat_sum = const.tile([128, KT], dt.float32)
            stat_sq = const.tile([128, KT], dt.float32)
            a_t = const.tile([128, KT], dt.float32)
            b_t = const.tile([128, KT], dt.float32)
            xres = {}

            # ---- phase 1: stream x on two queues + per-shard stats ---------
            def phase1(half):
                for t in range(KBASE[half], KBASE[half] + HKS[half]):
                    if t < NRES:
                        xs = xrp.tile([128, BS], dt.float32, name="xk")
                        xres[t] = xs
                    else:
                        xs = xsp.tile([128, BS], dt.float32, name="xs")
                    nc.sync.dma_start(xs[:], xt_d[t * 128:(t + 1) * 128, :])
                    nc.vector.reduce_sum(stat_sum[:, t:t + 1], xs[:],
                                         axis=mybir.AxisListType.X)
                    sq = sqdp.tile([128, BS], dt.float8e4, name="sq")
                    nc.scalar.activation(sq[:], xs[:], AF.Square,
                                         accum_out=stat_sq[:, t:t + 1])

            # ---- AllReduce staging/trigger, separate from coefficient math -
            # NOTE: a collective enqueued before the phase-1 DMAs stalls their
            # completions until the CC barrier clears (measured +40us), so no
            # priming op, and the gpsimd queue carries ONLY the two triggers
            # (a trigger blocks its queue until the collective finishes)
            # followed by the output drains.
            outbs = [None, None]

            def ar_stage(half):
                HK = HKS[half]
                ks = slice(KBASE[half], KBASE[half] + HK)
                inb = dram.tile([128, 2 * HK], dt.float32, name=f"inb{half}",
                                tag=f"inb{half}")
                outb = dram.tile([128, 2 * HK], dt.float32,
                                 name=f"outb{half}", tag=f"outb{half}")
                nc.scalar.dma_start(inb[:, 0:HK], stat_sum[:, ks])
                nc.scalar.dma_start(inb[:, HK:2 * HK], stat_sq[:, ks])
                nc.gpsimd.collective_compute(
                    "AllReduce", ALU.add,
                    replica_groups=[list(range(N_CORES))],
                    ins=[inb.opt()], outs=[outb.opt()],
                )
                outbs[half] = outb

            def bn_math(half):
                # a = gamma/std, b = beta - mean*a
                HK = HKS[half]
                k0 = KBASE[half]
                ks = slice(k0, k0 + HK)
                # read-back sits here (not in ar_stage) so its semaphore wait
                # never blocks the phase-1 sq chain on the ACT queue
                st = const.tile([128, 2 * HK], dt.float32,
                                name=f"sall{half}", tag=f"sall{half}")
                nc.scalar.dma_start(st[:], outbs[half][:])
                mean = const.tile([128, HK], dt.float32, name=f"mean{half}",
                                  tag=f"mean{half}")
                nc.vector.tensor_scalar_mul(mean[:], st[:, 0:HK], 1.0 / B)
                var = const.tile([128, HK], dt.float32, name=f"var{half}",
                                 tag=f"var{half}")
                nc.vector.tensor_scalar_mul(var[:], st[:, HK:2 * HK], 1.0 / B)
                msq = const.tile([128, HK], dt.float32, name=f"msq{half}",
                                 tag=f"msq{half}")
                nc.vector.tensor_mul(msq[:], mean[:], mean[:])
                nc.vector.tensor_sub(var[:], var[:], msq[:])
                std = const.tile([128, HK], dt.float32, name=f"std{half}",
                                 tag=f"std{half}")
                nc.scalar.activation(std[:], var[:], AF.Sqrt,
                                     bias=eps_t[:, 0:1], scale=1.0)
                ivs = const.tile([128, HK], dt.float32, name=f"ivs{half}",
                                 tag=f"ivs{half}")
                nc.vector.reciprocal(ivs[:], std[:])
                nc.vector.tensor_mul(a_t[:, ks], ivs[:], gb_t[:, ks])
                nc.vector.tensor_mul(b_t[:, ks], mean[:], a_t[:, ks])
                nc.vector.tensor_sub(
                    b_t[:, ks],
                    gb_t[:, KT + k0:KT + k0 + HK],
                    b_t[:, ks])

            phase1(0)
            ar_stage(0)
            phase1(1)

            # early reloads for the held chunk-A tail (12..15): issued on the
            # ACT queue before the AR read-backs so they never block behind
            # the collective's completion wait
            xphase2 = {}
            for t in range(NRES, FP8_KT):
                xs = xap.tile([128, BS], dt.float32, name=f"xa{t}",
                              tag=f"xa{t}", bufs=1)
                nc.scalar.dma_start(xs[:], xt_d[t * 128:(t + 1) * 128, :])
                xphase2[t] = xs

            ar_stage(1)
            bn_math(0)

            # ---- phase 2a: signs for chunk A (gates the fp8 pre-stage) -----
            hbpair = [hbpp.tile([128, 2 * BS], dt.float8e4, name=f"hbp{i}",
                                tag=f"hbp{i}") for i in range(NPAIR)]
            hbsing = [hbsp.tile([128, BS], dt.float8e4, name=f"hbs{i}",
                                tag=f"hbs{i}") for i in range(NB16)]
            for t in range(FP8_KT):
                xs = xres[t] if t in xres else xphase2[t]
                xphase2[t] = xs
                hb = hbpair[t // 2][:, (t % 2) * BS:(t % 2 + 1) * BS]
                nc.scalar.activation(hb, xs[:], AF.Sign,
                                     bias=b_t[:, t:t + 1],
                                     scale=a_t[:, t:t + 1])

            # ---- phase 3a: fp8 pair pre-stage for oq 0..PRE_OQ-1 -----------
            # Runs under the AR-B / chunk-B sign shadow; partials spill to
            # SBUF in bf16 and are re-added at drain time.
            def mm_pair(pt, w8t, kp, bt, start, stop):
                nc.tensor.matmul(
                    pt[:],
                    hbpair[kp][:].rearrange("p (i b) -> p i b", i=2)
                    [:, :, bt * 128:(bt + 1) * 128],
                    w8t[:].rearrange("p (i o) -> p i o", i=2),
                    start=start, stop=stop, perf_mode=PM.DoubleRow)

            def load_w8(oq):
                w8s = []
                for kp in range(NPAIR):
                    w8t = w8p.tile([128, 2 * 512], dt.float8e4, name="w8t")
                    nc.sync.dma_start(
                        w8t[:].rearrange("p (i o) -> p i o", i=2),
                        w8_d[kp * 128:(kp + 1) * 128, :]
                        .rearrange("p (i o) -> p i o", i=2)
                        [:, :, oq * 512:(oq + 1) * 512])
                    w8s.append(w8t)
                return w8s

            def load_w16(oq):
                w16s = []
                for j in range(NB16):
                    wtt = w16p.tile([128, 512], dt.bfloat16, name="wtt")
                    nc.sync.dma_start(
                        wtt[:],
                        w16_d[j * 128:(j + 1) * 128,
                              oq * 512:(oq + 1) * 512])
                    w16s.append(wtt)
                return w16s

            # Per-bt cyclic rotation of the k order spreads each W tile's
            # last-read across the oq span, so the next oq's W prefetch
            # trickles in instead of bursting at the boundary (which left
            # the PE idle long enough for HAM to re-throttle).
            partials = {}
            spills = []
            for oq in range(PRE_OQ):
                w8s = load_w8(oq)
                for bt in range(NBT):
                    pt = psp.tile([128, 512], dt.float32, name="pt", tag="pt")
                    for i in range(NPAIR):
                        kp = (bt + i) % NPAIR
                        mm_pair(pt, w8s[kp], kp, bt, i == 0, i == NPAIR - 1)
                    par = parp.tile([128, 512], dt.bfloat16,
                                    name=f"par{oq}_{bt}",
                                    tag=f"par{oq}_{bt}", bufs=1)
                    partials[(oq, bt)] = par
                    spills.append((par, pt))

            # DVE: spills for oq0 first (they recycle PSUM banks for the
            # pre-stage); the rest interleave with chunk-B |h| accumulation.
            for par, pt in spills[:NBT]:
                nc.vector.tensor_copy(par[:], pt[:])
            rest = list(spills[NBT:])

            bn_math(1)

            # w16 for oq0 must land ahead of the phase-2b x refetches that
            # share the sync queue (the main loop needs it right away).
            w16_oq0 = load_w16(0)

            # ---- phase 2b: chunk-B signs + all |h| accumulation ------------
            # ACT order: sign16, abs16, sign17, abs17, ... then abs for chunk
            # A at the end (resident/held tiles -- no buffer pressure).
            def acc_abs(t, xs):
                habs = habsp.tile([128, BS], dt.bfloat16, name="habs",
                                  tag="habs")
                nc.scalar.activation(habs[:], xs[:], AF.Abs,
                                     bias=b_t[:, t:t + 1],
                                     scale=a_t[:, t:t + 1])
                nc.vector.tensor_add(acc[:], acc[:], habs[:])

            for t in range(FP8_KT, KT):
                xs = xbp.tile([128, BS], dt.float32, name="xb")
                nc.sync.dma_start(xs[:], xt_d[t * 128:(t + 1) * 128, :])
                nc.scalar.activation(hbsing[t - FP8_KT][:], xs[:], AF.Sign,
                                     bias=b_t[:, t:t + 1],
                                     scale=a_t[:, t:t + 1])
                acc_abs(t, xs)
                # interleave ~1.5 pre-stage spills per chunk-B tile on DVE
                for _ in range(2 if t % 2 == 0 else 1):
                    if rest:
                        par, pt = rest.pop(0)
                        nc.vector.tensor_copy(par[:], pt[:])
            for par, pt in rest:
                nc.vector.tensor_copy(par[:], pt[:])
            for t in range(FP8_KT):
                acc_abs(t, xphase2[t])

            # ---- main matmul stream ---------------------------------------
            # oq < PRE_OQ: bf16 k-tiles only + partial re-add; oq >= PRE_OQ:
            # full 8-pair + 16-single accumulation.  Beta's two ones-matmuls
            # slot in after oq0 (acc is complete by then); oq0's drains are
            # deferred-scaled once betaT lands.
            beta_ready = False
            betaT = None
            deferred = []
            for oq in range(NOQ):
                # w8 ahead of w16 on the queue: the pair matmuls consume it
                # first at each oq start
                w8s = None if oq < PRE_OQ else load_w8(oq)
                w16s = w16_oq0 if oq == 0 else load_w16(oq)
                for bt in range(NBT):
                    pt = psp.tile([128, 512], dt.float32, name="pt", tag="pt")
                    if oq < PRE_OQ:
                        for i in range(NB16):
                            j = (2 * bt + i) % NB16
                            nc.tensor.matmul(
                                pt[:], hbsing[j][:, bt * 128:(bt + 1) * 128],
                                w16s[j][:], start=(i == 0),
                                stop=(i == NB16 - 1))
                    else:
                        for i in range(NPAIR):
                            kp = (bt + i) % NPAIR
                            mm_pair(pt, w8s[kp], kp, bt, i == 0, False)
                        for i in range(NB16):
                            j = (2 * bt + i) % NB16
                            nc.tensor.matmul(
                                pt[:], hbsing[j][:, bt * 128:(bt + 1) * 128],
                                w16s[j][:], start=False,
                                stop=(i == NB16 - 1))
                    if not beta_ready:
                        # beta not ready yet: drain unscaled, scale later
                        ybu = ybp.tile([128, 512], dt.float32, name="ybu",
                                       tag="yb")
                        nc.vector.tensor_add(ybu[:], pt[:],
                                             partials[(oq, bt)][:])
                        deferred.append((oq, bt, ybu))
                    else:
                        yb = ybp.tile([128, 512], dt.float32, name="yb",
                                      tag="yb")
                        if oq < PRE_OQ:
                            tmp = ybp.tile([128, 512], dt.float32, name="tmp",
                                           tag="yb")
                            nc.vector.tensor_add(tmp[:], pt[:],
                                                 partials[(oq, bt)][:])
                            nc.vector.tensor_scalar_mul(
                                yb[:], tmp[:], betaT[:, bt:bt + 1])
                        else:
                            nc.vector.tensor_scalar_mul(
                                yb[:], pt[:], betaT[:, bt:bt + 1])
                        nc.scalar.dma_start(
                            out_d[bt * 128:(bt + 1) * 128,
                                  oq * 512:(oq + 1) * 512], yb[:])

                if oq == 0:
                    # ---- beta: ones-matmul over the |h| accumulator --------
                    # the two 512-col halves sit on partitions 0 and 32 of
                    # ONE psum bank ([1, 1024] would span two banks)
                    beta_ps = psp.tile([33, 512], dt.float32, tag="beta",
                                       bufs=1)
                    for half in range(BS // 512):
                        nc.tensor.matmul(
                            beta_ps[half * 32:half * 32 + 1, :], ones[:],
                            acc[:, half * 512:(half + 1) * 512],
                            start=True, stop=True)
                    beta_sb = const.tile([33, 512], dt.float32)
                    nc.vector.tensor_scalar_mul(beta_sb[:], beta_ps[:],
                                                1.0 / (D * W_SCALE))
                    # [1, BS] -> [128, BS/128] transpose via a DRAM bounce
                    # (DRAM-side access patterns are unconstrained)
                    bb = dram.tile([1, BS], dt.float32)
                    nc.scalar.dma_start(bb[:, 0:512], beta_sb[0:1, :])
                    nc.scalar.dma_start(bb[:, 512:BS], beta_sb[32:33, :])
                    betaT = const.tile([128, BS // 128], dt.float32)
                    nc.scalar.dma_start(
                        betaT[:], bb.rearrange("o (j p) -> (o p) j", p=128))
                    beta_ready = True
                    # late scale + writeback for the deferred oq0 drains
                    for doq, dbt, ybu in deferred:
                        yb = ybp.tile([128, 512], dt.float32, name="yb2",
                                      tag="yb")
                        nc.vector.tensor_scalar_mul(
                            yb[:], ybu[:], betaT[:, dbt:dbt + 1])
                        nc.scalar.dma_start(
                            out_d[dbt * 128:(dbt + 1) * 128,
                                  doq * 512:(doq + 1) * 512], yb[:])

    nc.compile()
    return nc


def kernel(x, bn_gamma, bn_beta, W, alpha):
    global _nc_cache, LAST_RESULT
    x = np.ascontiguousarray(x, dtype=np.float32)
    W = np.asarray(W, dtype=np.float32)
    alpha = np.asarray(alpha, dtype=np.float32)

    # host prep: fold alpha into W, transpose to [in, out], upscale by 2^12
    ws = np.ascontiguousarray((W * alpha[:, None]).T) * np.float32(W_SCALE)
    # fp8 part: k-tiles 0..15 -> pair layout [kp*128+p, i*D+o],
    # value = ws[(2kp + i)*128 + p, o]
    w8 = ws[:FP8_KT * 128].reshape(NPAIR, 2, 128, D).transpose(0, 2, 1, 3)
    w8 = np.clip(w8, -240.0, 240.0).reshape(NPAIR * 128, 2 * D)
    w8 = np.ascontiguousarray(w8).astype(ml_dtypes.float8_e4m3)
    # bf16 part: k-tiles 16..31 (same 2^12 scale -- exact in bf16)
    w16 = np.ascontiguousarray(ws[FP8_KT * 128:]).astype(ml_dtypes.bfloat16)
    # gamma/beta in per-partition layout: gb[p, t] = gamma[t*128 + p]
    gb = np.concatenate(
        [np.asarray(bn_gamma, np.float32).reshape(KT, 128).T,
         np.asarray(bn_beta, np.float32).reshape(KT, 128).T], axis=1)
    gb = np.ascontiguousarray(gb)

    if _nc_cache is None:
        _nc_cache = _build()
    nc = _nc_cache

    in_maps = []
    for c in range(N_CORES):
        xT = np.ascontiguousarray(x[c * BS:(c + 1) * BS, :].T)
        in_maps.append({"xt": xT, "w8": w8, "w16": w16, "gb": gb})

    res = run_bass_kernel_spmd(nc, in_maps, core_ids=list(range(N_CORES)),
                               trace=TRACE)
    LAST_RESULT = res
    return np.concatenate([res.results[c]["out"] for c in range(N_CORES)],
                          axis=0)
